# revision 1
# baseline (speedup 1.0000x reference)
"""Trainium2 Bass kernel for pairwise diagonal-Gaussian KL energies.

energies[b, i] = 0.5 * sum_d [ log(d_id) + (1 + (x_bd - mu_id)^2) / d_id - 1 ]
with d = clip(diag, 1e-6),  x: (4096, 128), mean/diag: (8192, 128).

Sharding: tensor-parallel over codebook rows (n_in) across 8 cores.
Each core gets the full x (host-transposed to [dim, batch], cast bf16) and a
1024-row shard of mean/diag (host-transposed, f32), and produces the
(batch, 1024) column slab of the output; the host concatenates the slabs.

Per-core device pipeline (everything in [dim(partition), *] layout):
  inv    = exp(-ln(max(diag, 1e-6)))           ScalarE (DVE divide is slow)
  minvb  = bf16(-mean * inv), invb = bf16(inv) DVE/GpSimd
  xxb    = bf16(0.5 * x^2) = (x*0.5)*x         DVE (no ACT Square table load)
  cvec   = 0.5*colsum(inv*(1+mean^2) + ln d) - dim/2   PE ones-column matmul
  cb     = cvec broadcast to 128 partitions    PE K=1 ones-row matmul (f32)
  prep is pipelined in column halves and input DMAs split across both
  HWDGE rings (diag first -- it heads the dependency chain)
  per 128-batch tile: PSUM[128,1024] = xxb.T@invb + xb.T@minvb (4 bf16
  matmuls, N=512 each, fp32 accumulate; fp32 matmuls are ~4x slower per
  column on trn2 PE, so everything streams bf16), evacuated by two
  [128,512] DVE tensor_adds (+cb, fusing the per-codebook constant) into
  SBUF f32, then one 512 KiB HWDGE DMA per tile.

Measured (8x trn2 NC, wall-clock slope over in-kernel For_i repeats):
~54.5 us per full 32-tile pass vs a ~51 us pure-DMA floor for the 16 MiB
f32 output slab (~330 GB/s/core) -- i.e. ~94% of the output-write
roofline; PE 30 us, DVE 46 us, all hidden under the DMA. One-time prep
~22 us (cost model; table load + input DMA + codebook chain), total
single-shot ~77 us.  Max relative error vs the f32 jax reference:
2.1e-3 (from the bf16 GEMM operands).
Ablations tried and rejected: dual HWDGE rings (no gain), grouped 1-4 MiB
output DMAs (no gain), ScalarE/GpSimd evacuation offload (slower), bf16
output (42.7 us but quantizes the result, 5.2e-3), full-width [128,1024]
cross-bank PSUM evacuation with f32 output (intermittent device crash).
"""

import numpy as np

N_IN, DIM, BATCH = 8192, 128, 4096
N_CORES = 8
SHARD = N_IN // N_CORES  # 1024 codebook rows per core
PD_THR = 1e-6
BT = BATCH // 128  # 32 batch tiles per core

_BUILD_CACHE = {}


def build(
    repeat=1,
    psum_bufs=3,
    out_bufs=4,
    out_group=1,
    out_dma_engines=("sync",),
    skip_mm=False,
    skip_evac=False,
    skip_out_dma=False,
    evac_full=False,
    use_stt=True,
    out_dtype="f32",
    act_tiles=0,
    gp_tiles=0,
):
    """Build + compile the single-core SPMD program. Cached per config.

    act_tiles: number of batch tiles (of 32) whose PSUM gets the constant
    via a K=2 bf16 ones-matmul pre-bias and is evacuated by ScalarE as a
    pure copy; the rest are evacuated by DVE tensor_add(+cb). Balances DVE
    against ScalarE+PE when the out-DMA is no longer the bottleneck.
    """
    key = (
        repeat, psum_bufs, out_bufs, out_group, out_dma_engines,
        skip_mm, skip_evac, skip_out_dma, evac_full, use_stt,
        out_dtype, act_tiles, gp_tiles,
    )
    if key in _BUILD_CACHE:
        return _BUILD_CACHE[key]

    import contextlib

    import concourse.bass as bass
    import concourse.bacc as bacc
    import concourse.tile as tile
    import concourse.mybir as mybir

    f32 = mybir.dt.float32
    bf16 = mybir.dt.bfloat16
    AF = mybir.ActivationFunctionType
    ALU = mybir.AluOpType

    nc = bacc.Bacc("TRN2", target_bir_lowering=False, debug=False)

    odt = f32 if out_dtype == "f32" else bf16
    xb_d = nc.dram_tensor("xb", [DIM, BATCH], bf16, kind="ExternalInput")
    mt_d = nc.dram_tensor("meant", [DIM, SHARD], f32, kind="ExternalInput")
    dg_d = nc.dram_tensor("diagt", [DIM, SHARD], f32, kind="ExternalInput")
    out_d = nc.dram_tensor("out", [BATCH, SHARD], odt, kind="ExternalOutput")
    out_ap = out_d.ap()
    G = out_group
    # [BT/G, 128, G*SHARD] view: dma tile ibg covers b-rows [ibg*128G,
    # (ibg+1)*128G) as G free-dim-concatenated blocks.
    out_gv = out_ap.rearrange("(n g p) i -> n p g i", g=G, p=128)

    with tile.TileContext(nc) as tc:
        with (
            tc.tile_pool(name="persist", bufs=1) as pp,
            tc.tile_pool(name="prep", bufs=1) as prep,
        ):
            # ---- loads: diag heads the dependency chain, so it goes
            # first on the sync ring; mean rides the scalar ring; x halves
            # split across both ----
            dg = prep.tile([DIM, SHARD], f32)
            nc.sync.dma_start(dg[:], dg_d.ap())
            mt = prep.tile([DIM, SHARD], f32)
            nc.scalar.dma_start(mt[:], mt_d.ap())
            xb = pp.tile([DIM, BATCH], bf16)
            xhalf = BATCH // 2
            nc.sync.dma_start(xb[:, :xhalf], xb_d.ap()[:, :xhalf])
            nc.scalar.dma_start(xb[:, xhalf:], xb_d.ap()[:, xhalf:])

            zb = pp.tile([DIM, 1], f32)
            nc.gpsimd.memset(zb[:], 0.0)
            half_col = pp.tile([DIM, 1], f32)  # 0.5-valued: colsum * 0.5
            nc.gpsimd.memset(half_col[:], 0.5)
            ones_row = pp.tile([1, DIM], f32)  # K=1 broadcast stationary
            nc.gpsimd.memset(ones_row[:], 1.0)

            # ---- codebook prep, pipelined in column halves; everything
            # the first batch tiles need (invb/minvb/xxb-half/cb-half) is
            # emitted before any half-1 work so the per-engine FIFOs let
            # the main loop's output-DMA stream start early ----
            dc = prep.tile([DIM, SHARD], f32)
            lg = prep.tile([DIM, SHARD], f32)
            inv = prep.tile([DIM, SHARD], f32)
            invb = pp.tile([DIM, SHARD], bf16)
            minvb = pp.tile([DIM, SHARD], bf16)
            m2 = prep.tile([DIM, SHARD], f32)
            t2 = prep.tile([DIM, SHARD], f32)
            s2 = prep.tile([DIM, SHARD], f32)
            cvec = pp.tile([1, SHARD], f32)
            xxb = pp.tile([DIM, BATCH], bf16)
            cb = pp.tile([DIM, SHARD], f32)
            with (
                tc.tile_pool(
                    name="psum_prep", bufs=1, space=bass.MemorySpace.PSUM
                ) as psp,
                tc.tile_pool(
                    name="psum_prep2", bufs=1, space=bass.MemorySpace.PSUM
                ) as psp2,
            ):
                cps = psp.tile([1, SHARD], f32)
                bps = psp2.tile([DIM, SHARD], f32)
                xh = BATCH // 2
                for h in range(SHARD // 512):
                    sl = slice(h * 512, (h + 1) * 512)
                    nc.vector.tensor_scalar_max(dc[:, sl], dg[:, sl], PD_THR)
                    nc.scalar.activation(lg[:, sl], dc[:, sl], AF.Ln, bias=zb[:])
                    nc.scalar.activation(
                        inv[:, sl], lg[:, sl], AF.Exp, bias=zb[:], scale=-1.0
                    )
                    nc.vector.tensor_mul(m2[:, sl], mt[:, sl], mt[:, sl])
                    nc.gpsimd.tensor_copy(invb[:, sl], inv[:, sl])
                    nc.vector.scalar_tensor_tensor(
                        minvb[:, sl], mt[:, sl], -1.0, inv[:, sl],
                        ALU.mult, ALU.mult,
                    )
                    nc.vector.scalar_tensor_tensor(
                        t2[:, sl], m2[:, sl], 1.0, inv[:, sl], ALU.add, ALU.mult
                    )
                    nc.vector.tensor_add(s2[:, sl], t2[:, sl], lg[:, sl])
                    nc.tensor.matmul(cps[:, sl], half_col[:], s2[:, sl])
                    nc.scalar.activation(
                        cvec[:, sl], cps[:, sl], AF.Copy, bias=-float(DIM // 2)
                    )
                    # xxb = bf16(0.5 x^2) = (x*0.5)*x on DVE (no ACT table)
                    cs = slice(h * xh, (h + 1) * xh)
                    nc.vector.scalar_tensor_tensor(
                        xxb[:, cs], xb[:, cs], 0.5, xb[:, cs],
                        ALU.mult, ALU.mult,
                    )
                    # cb = cvec broadcast to 128 partitions for this half
                    nc.tensor.matmul(bps[:, sl], ones_row[:], cvec[:, sl])
                    nc.vector.tensor_copy(cb[:, sl], bps[:, sl])

            if act_tiles:
                # split cvec into bf16 + bf16 residual rows for an exact
                # K=2 ones-matmul PSUM pre-bias (ScalarE-evacuated tiles)
                cvec_b = prep.tile([1, SHARD], bf16)
                nc.vector.tensor_copy(cvec_b[:], cvec[:])
                cvec_bf = prep.tile([1, SHARD], f32)
                nc.vector.tensor_copy(cvec_bf[:], cvec_b[:])
                cres = prep.tile([1, SHARD], f32)
                nc.vector.tensor_sub(cres[:], cvec[:], cvec_bf[:])
                cvr = pp.tile([2, SHARD], bf16)
                nc.gpsimd.dma_start(cvr[0:1, :], cvec_b[:])
                nc.gpsimd.dma_start(cvr[1:2, :], cres[:])  # SWDGE casts f32->bf16
                ones2 = pp.tile([2, DIM], bf16)
                nc.gpsimd.memset(ones2[:], 1.0)

            # ---- main loop ----
            with (
                tc.tile_pool(
                    name="psum", bufs=psum_bufs, space=bass.MemorySpace.PSUM
                ) as psm,
                tc.tile_pool(name="outs", bufs=out_bufs) as osp,
            ):
                act_set = (
                    {int(i * BT / act_tiles) for i in range(act_tiles)}
                    if act_tiles
                    else set()
                )
                gp_set = (
                    {i for i in range(BT) if i not in act_set}
                    if gp_tiles
                    else set()
                )
                gp_set = set(sorted(gp_set)[:gp_tiles])
                loop_cm = (
                    tc.For_i(0, repeat, 1) if repeat > 1 else contextlib.nullcontext()
                )
                with loop_cm:
                    for ibg in range(BT // G):
                        ob = osp.tile([128, G * SHARD], odt)
                        for g in range(G):
                            ib = ibg * G + g
                            bs = slice(ib * 128, (ib + 1) * 128)
                            gs = slice(g * SHARD, (g + 1) * SHARD)
                            i0 = slice(0, 512)
                            i1 = slice(512, 1024)
                            on_act = ib in act_set
                            ps = psm.tile([128, SHARD], f32)
                            if not skip_mm:
                                if on_act:
                                    nc.tensor.matmul(
                                        ps[:, i0], ones2[:], cvr[:, i0],
                                        start=True, stop=False,
                                    )
                                    nc.tensor.matmul(
                                        ps[:, i1], ones2[:], cvr[:, i1],
                                        start=True, stop=False,
                                    )
                                nc.tensor.matmul(
                                    ps[:, i0], xxb[:, bs], invb[:, i0],
                                    start=not on_act, stop=False,
                                )
                                nc.tensor.matmul(
                                    ps[:, i1], xxb[:, bs], invb[:, i1],
                                    start=not on_act, stop=False,
                                )
                                nc.tensor.matmul(
                                    ps[:, i0], xb[:, bs], minvb[:, i0],
                                    start=False, stop=True,
                                )
                                nc.tensor.matmul(
                                    ps[:, i1], xb[:, bs], minvb[:, i1],
                                    start=False, stop=True,
                                )
                            if not skip_evac:
                                if evac_full:
                                    src = ps[:] if not skip_mm else cb[:]
                                    if on_act:
                                        nc.scalar.copy(ob[:, gs], src)
                                    elif ib in gp_set:
                                        stage = osp.tile(
                                            [128, SHARD], f32, tag="gpstage"
                                        )
                                        nc.scalar.copy(stage[:], src)
                                        nc.gpsimd.tensor_add(
                                            ob[:, gs], stage[:], cb[:]
                                        )
                                    else:
                                        nc.vector.tensor_add(ob[:, gs], src, cb[:])
                                else:
                                    for h in (i0, i1):
                                        hs = slice(
                                            g * SHARD + h.start, g * SHARD + h.stop
                                        )
                                        src = (
                                            ps[:, h] if not skip_mm else cb[:, h]
                                        )
                                        if on_act:
                                            nc.scalar.copy(ob[:, hs], src)
                                        else:
                                            nc.vector.tensor_add(
                                                ob[:, hs], src, cb[:, h]
                                            )
                        if not skip_out_dma:
                            eng = getattr(
                                nc, out_dma_engines[ibg % len(out_dma_engines)]
                            )
                            dummy = cb if odt is not bf16 else invb
                            src = ob[:] if not skip_evac else dummy[:]
                            if G == 1:
                                eng.dma_start(out_ap[ibg * 128 : ibg * 128 + 128, :], src)
                            else:
                                src = src.rearrange("p (g i) -> p g i", g=G)
                                eng.dma_start(out_gv[ibg], src)

    nc.compile()
    _BUILD_CACHE[key] = nc
    return nc


def make_in_maps(x, mean, diag):
    import ml_dtypes

    xb = np.ascontiguousarray(
        np.asarray(x).T.astype(ml_dtypes.bfloat16)
    )
    in_maps = []
    for c in range(N_CORES):
        sl = slice(c * SHARD, (c + 1) * SHARD)
        in_maps.append(
            {
                "xb": xb,
                "meant": np.ascontiguousarray(
                    np.asarray(mean)[sl].T.astype(np.float32, copy=False)
                ),
                "diagt": np.ascontiguousarray(
                    np.asarray(diag)[sl].T.astype(np.float32, copy=False)
                ),
            }
        )
    return in_maps


def kernel(x, mean, diag):
    from concourse.bass_utils import run_bass_kernel_spmd

    nc = build(repeat=1)
    in_maps = make_in_maps(x, mean, diag)
    try:
        res = run_bass_kernel_spmd(nc, in_maps, list(range(N_CORES)))
    except Exception:
        # rare transient device error; one retry
        res = run_bass_kernel_spmd(nc, in_maps, list(range(N_CORES)))
    return np.concatenate(
        [res.results[c]["out"].astype(np.float32) for c in range(N_CORES)], axis=1
    )



# revision 10
# speedup vs baseline: 1.4245x; 1.4245x over previous
"""Trainium2 Bass kernel for pairwise diagonal-Gaussian KL energies.

energies[b, i] = 0.5 * sum_d [ log(d_id) + (1 + (x_bd - mu_id)^2) / d_id - 1 ]
with d = clip(diag, 1e-6),  x: (4096, 128), mean/diag: (8192, 128).

Sharding: tensor-parallel over codebook rows (n_in) across 8 cores.
Each core gets the full x (host-transposed to [dim, batch], cast bf16) and a
1024-row shard of mean/diag (host-transposed, cast bf16), and produces the
(1024, batch) row slab of the transposed output in fp16; the host
concatenates along n_in, transposes back to (batch, n_in), casts f32.

Design ([n_in(partition), batch(free)] output orientation):
  - PSUM tiles are [128 n_in, 512 batch]: stationary = codebook tile
    (invb/minvb [dim, 128] bf16), moving = batch data (xxb/xb [dim, 512]
    bf16).  Per 128-row i-tile: 8 matmuls with invb_it (start) + 8 with
    minvb_it (accumulate) -> 2 stationary swaps per i-tile, PE streams
    1 col/cycle bf16: 65.5k cycles @2.4GHz = 27.3 us/pass (bottleneck).
  - The per-codebook constant is a PER-PARTITION [128,1] vector here, so
    evacuation fuses it for free: ACT Identity(ps + bias) / DVE
    tensor_scalar_add alternating per 512-chunk; no extra PE work.
  - fp16 output: rel-err budget is 2e-2, fp16 adds ~2e-4; output DMA
    halves vs f32 to ~25 us/core (f32 was DMA-bound at ~51 us).
  - inv carries the 0.5 quad scale: inv_half = recip_approx_fast(2*clip(d))
    so invb = bf16(0.5/d), xxb = plain x^2 (fast TensorTensor / ACT
    Square), minvb = -2*mu*inv_half = -mu/d, and the ln(2) offset
    constant-folds into the colsum bias (-64*(1+ln2)).
  - prep: diag/mean ship as bf16 (halved DMA), both ACT tables warm at
    t=0, codebook chain in 512-col halves spread over DVE/ACT/Pool,
    colsum via 8 tiny N=1 matmuls (stationary = s2 i-tile), one DVE
    tensor_scalar_add finalizes cvt [128,8].
"""

import numpy as np

N_IN, DIM, BATCH = 8192, 128, 4096
N_CORES = 8
SHARD = N_IN // N_CORES  # 1024 codebook rows per core
PD_THR = 1e-6
IT = SHARD // 128  # 8 i-tiles of 128 codebook rows
BC = BATCH // 512  # 8 batch chunks of 512 per i-tile
CVT_BIAS = -float(DIM // 2) * (1.0 + float(np.log(2.0)))  # -64*(1+ln2)

_BUILD_CACHE = {}


def build(
    repeat=1,
    psum_bufs=8,
    out_bufs=3,
    out_dma_engines=("sync",),
    evac_pattern="AVAVAVAV",  # per batch-chunk engine: A=ACT, V=DVE, P=Pool
    skip_mm=False,
    skip_evac=False,
    skip_out_dma=False,
    out_dtype="f16",
):
    """Build + compile the single-core SPMD program. Cached per config."""
    key = (
        repeat, psum_bufs, out_bufs, out_dma_engines, evac_pattern,
        skip_mm, skip_evac, skip_out_dma, out_dtype,
    )
    if key in _BUILD_CACHE:
        return _BUILD_CACHE[key]

    import contextlib

    import concourse.bass as bass
    import concourse.bacc as bacc
    import concourse.tile as tile
    import concourse.mybir as mybir

    f32 = mybir.dt.float32
    bf16 = mybir.dt.bfloat16
    f16 = mybir.dt.float16
    AF = mybir.ActivationFunctionType
    ALU = mybir.AluOpType

    nc = bacc.Bacc("TRN2", target_bir_lowering=False, debug=False)

    odt = {"f16": f16, "f32": f32, "bf16": bf16}[out_dtype]
    xb_d = nc.dram_tensor("xb", [DIM, BATCH], bf16, kind="ExternalInput")
    mt_d = nc.dram_tensor("meant", [DIM, SHARD], bf16, kind="ExternalInput")
    dg_d = nc.dram_tensor("diagt", [DIM, SHARD], bf16, kind="ExternalInput")
    out_d = nc.dram_tensor("out", [SHARD, BATCH], odt, kind="ExternalOutput")
    out_ap = out_d.ap()

    with tile.TileContext(nc) as tc:
        with (
            tc.tile_pool(name="persist", bufs=1) as pp,
            tc.tile_pool(name="prep", bufs=1) as prep,
        ):
            zb = pp.tile([DIM, 1], f32)
            nc.gpsimd.memset(zb[:], 0.0)
            ones_col = pp.tile([DIM, 1], f32)
            nc.gpsimd.memset(ones_col[:], 1.0)

            # warm BOTH ACT tables (Square -> set0, Ln -> natural_log)
            # before anything else queues on the ACT sequencer, so both
            # ATLs overlap the input-DMA window
            warm = prep.tile([DIM, 1], f32)
            nc.scalar.activation(warm[:], zb[:], AF.Square, bias=zb[:])
            nc.scalar.activation(warm[:], zb[:], AF.Ln, bias=zb[:])

            # ---- input loads, all on the (otherwise idle) sync ring; the
            # transfers serialize on the DMA queue anyway, so order =
            # dependency order: diag, mean, then x (half1 first: ACT's
            # Square chunks read it).
            dg = prep.tile([DIM, SHARD], bf16)
            mt = prep.tile([DIM, SHARD], bf16)
            xb = pp.tile([DIM, BATCH], bf16)
            xhalf = BATCH // 2
            nc.sync.dma_start(dg[:], dg_d.ap())
            nc.sync.dma_start(mt[:], mt_d.ap())
            nc.sync.dma_start(xb[:, xhalf:], xb_d.ap()[:, xhalf:])
            nc.sync.dma_start(xb[:, :xhalf], xb_d.ap()[:, :xhalf])

            # ---- codebook prep in 512-col halves; xxb in 1024-col chunks
            dc = prep.tile([DIM, SHARD], f32)   # 2*clip(diag)
            lg = prep.tile([DIM, SHARD], f32)   # ln(2*clip(diag))
            inv = prep.tile([DIM, SHARD], f32)  # 0.5/clip(diag)
            invb = pp.tile([DIM, SHARD], bf16)
            minvb = pp.tile([DIM, SHARD], bf16)
            m2 = prep.tile([DIM, SHARD], f32)
            t2 = prep.tile([DIM, SHARD], f32)
            s2 = prep.tile([DIM, SHARD], f32)
            xxb = pp.tile([DIM, BATCH], bf16)
            cvt = pp.tile([DIM, IT], f32)  # per-codebook constant, [128, 8]
            with tc.tile_pool(
                name="psum_prep", bufs=1, space=bass.MemorySpace.PSUM
            ) as psp:
                cps = psp.tile([DIM, IT], f32)
                XQ = BATCH // 4  # 1024-col x^2 chunks
                H = [slice(0, 512), slice(512, 1024)]
                XC = [slice(i * XQ, (i + 1) * XQ) for i in range(4)]
                # Per-engine FIFOs, emitted in intended execution order.
                # DVE: dc -> recip -> invb -> minvb -> s2, then one x^2 chunk
                for sl in H:
                    nc.vector.tensor_scalar(
                        dc[:, sl], dg[:, sl], PD_THR, 2.0, ALU.max, ALU.mult
                    )
                for sl in H:
                    nc.vector.reciprocal_approx_fast(inv[:, sl], dc[:, sl])
                # ACT: ln(2*clip(diag)), then squares (x half1 lands first)
                for sl in H:
                    nc.scalar.activation(lg[:, sl], dc[:, sl], AF.Ln, bias=zb[:])
                # Pool: m2 + invb cast (TT/copy only — Pool codegen
                # rejects scalar_tensor_tensor)
                for sl in H:
                    nc.gpsimd.tensor_mul(m2[:, sl], mt[:, sl], mt[:, sl])
                for sl in H:
                    nc.gpsimd.tensor_copy(invb[:, sl], inv[:, sl])
                for sl in H:
                    nc.vector.scalar_tensor_tensor(
                        minvb[:, sl], mt[:, sl], -2.0, inv[:, sl],
                        ALU.mult, ALU.mult,
                    )
                for sl in H:
                    nc.vector.scalar_tensor_tensor(
                        t2[:, sl], m2[:, sl], 1.0, inv[:, sl], ALU.add, ALU.mult
                    )
                for cs in (XC[2], XC[3], XC[0]):
                    nc.scalar.activation(
                        xxb[:, cs], xb[:, cs], AF.Square, bias=zb[:]
                    )
                # s2 = 0.5*lg + t2; colsum via N=1 matmul per i-tile
                for h, sl in enumerate(H):
                    nc.vector.scalar_tensor_tensor(
                        s2[:, sl], lg[:, sl], 0.5, t2[:, sl], ALU.mult, ALU.add
                    )
                    for it in range(h * (IT // 2), (h + 1) * (IT // 2)):
                        isl = slice(it * 128, (it + 1) * 128)
                        nc.tensor.matmul(
                            cps[:, it : it + 1], s2[:, isl], ones_col[:]
                        )
                nc.vector.tensor_mul(xxb[:, XC[1]], xb[:, XC[1]], xb[:, XC[1]])
                # cvt = cps - 64*(1+ln2), one op over [128, 8]
                nc.vector.tensor_scalar_add(cvt[:], cps[:], CVT_BIAS)

            # ---- main loop ----
            with (
                tc.tile_pool(
                    name="psum", bufs=psum_bufs, space=bass.MemorySpace.PSUM
                ) as psm,
                tc.tile_pool(name="outs", bufs=out_bufs) as osp,
            ):
                loop_cm = (
                    tc.For_i(0, repeat, 1) if repeat > 1 else contextlib.nullcontext()
                )
                with loop_cm:
                    for it in range(IT):
                        isl = slice(it * 128, (it + 1) * 128)
                        ob = osp.tile([128, BATCH], odt)
                        pss = []
                        if not skip_mm:
                            for b in range(BC):
                                bs = slice(b * 512, (b + 1) * 512)
                                ps = psm.tile([128, 512], f32)
                                nc.tensor.matmul(
                                    ps[:], invb[:, isl], xxb[:, bs],
                                    start=True, stop=False,
                                )
                                pss.append(ps)
                            for b in range(BC):
                                bs = slice(b * 512, (b + 1) * 512)
                                nc.tensor.matmul(
                                    pss[b][:], minvb[:, isl], xb[:, bs],
                                    start=False, stop=True,
                                )
                                if not skip_evac:
                                    e = evac_pattern[b % len(evac_pattern)]
                                    if e == "A":
                                        nc.scalar.activation(
                                            ob[:, bs], pss[b][:], AF.Identity,
                                            bias=cvt[:, it : it + 1],
                                        )
                                    elif e == "V":
                                        nc.vector.tensor_scalar_add(
                                            ob[:, bs], pss[b][:],
                                            cvt[:, it : it + 1],
                                        )
                                    else:
                                        nc.gpsimd.tensor_scalar_add(
                                            ob[:, bs], pss[b][:],
                                            cvt[:, it : it + 1],
                                        )
                        elif not skip_evac:
                            for b in range(BC):
                                bs = slice(b * 512, (b + 1) * 512)
                                nc.vector.tensor_scalar_add(
                                    ob[:, bs], xxb[:, bs], cvt[:, it : it + 1]
                                )
                        if not (skip_out_dma or skip_evac):
                            eng = getattr(
                                nc, out_dma_engines[it % len(out_dma_engines)]
                            )
                            eng.dma_start(out_ap[isl, :], ob[:])

    nc.compile()
    _BUILD_CACHE[key] = nc
    return nc


def make_in_maps(x, mean, diag):
    import ml_dtypes

    bf = ml_dtypes.bfloat16
    xb = np.ascontiguousarray(np.asarray(x).T.astype(bf))
    in_maps = []
    for c in range(N_CORES):
        sl = slice(c * SHARD, (c + 1) * SHARD)
        in_maps.append(
            {
                "xb": xb,
                "meant": np.ascontiguousarray(np.asarray(mean)[sl].T.astype(bf)),
                "diagt": np.ascontiguousarray(np.asarray(diag)[sl].T.astype(bf)),
            }
        )
    return in_maps


def kernel(x, mean, diag):
    from concourse.bass_utils import run_bass_kernel_spmd

    nc = build(repeat=1)
    in_maps = make_in_maps(x, mean, diag)
    try:
        res = run_bass_kernel_spmd(nc, in_maps, list(range(N_CORES)))
    except Exception:
        # rare transient device error; one retry
        res = run_bass_kernel_spmd(nc, in_maps, list(range(N_CORES)))
    # per-core out is (SHARD, BATCH) = energies.T slab; stack along n_in,
    # transpose back to (batch, n_in), cast to f32
    full_t = np.concatenate(
        [np.asarray(res.results[c]["out"]) for c in range(N_CORES)], axis=0
    )
    return np.ascontiguousarray(full_t.T).astype(np.float32)
